# revision 11
# baseline (speedup 1.0000x reference)
"""Trainium2 Bass kernel for the Performer-style random-feature map:

    out[n, s] = exp(-||x_n||^2 / 2) * S^{-1/2} * exp((x @ W.T)[n, s] + b[s])
              = exp((x @ W.T)[n, s] - 0.5*||x_n||^2 - 0.5*ln(S)) * exp(b[s])

Sharding: data-parallel over the N (row) axis across 8 NeuronCores; W and b
replicated.  Each core computes a [2048, 2048] output block.  Pure SPMD, no
collectives.

Per-core structure (sizes hardcoded for N=16384, D=1024, S=2048):
  - x^T and W^T live in SBUF as fp8(e4m3) [128, 8, *] k-strip stacks; the
    matmul contracts 256 elements per instruction via DoubleRow perf mode
    (2 fp8 weights per PE cell -> ~1.5x bf16 throughput).  W is pre-scaled
    by 16 on the host so its values sit in the fp8 normal range; the 1/16
    is folded into the ACT exp scale.  The k loop is outside the column
    loop so one stationary x block serves 4 matmuls (amortizes LDWEIGHTS).
  - natural-layout x rows stream in as fp16 per 128-row block; one DVE
    tensor_tensor_reduce computes bias_n = -0.5*||x_n||^2 - 0.5*ln(S).
  - per row block: 2x [128, 1024] PSUM groups -> ACT exp(psum/16 + bias_n)
    in bf16 -> DVE bf16 multiply by exp(b) broadcast (2x packed mode) ->
    DMA out in bf16 (host upcasts to f32; the tolerance and the actual
    output range make this exact here).
"""

import sys
from contextlib import ExitStack

if "/opt/trn_rl_repo" not in sys.path:
    sys.path.insert(0, "/opt/trn_rl_repo")

import numpy as np

import concourse.bacc as bacc
import concourse.bass as bass
import concourse.tile as tile
from concourse import mybir

P = 128          # SBUF partitions
N_FULL = 16384   # total rows
D_FULL = 1024    # contraction dim
S_FULL = 2048    # output features
N_CORES = 8
NC_FULL = N_FULL // N_CORES  # rows per core

W_SCALE = 16.0   # host-side W multiplier (keeps fp8 W in the normal range)

F32 = mybir.dt.float32
F16 = mybir.dt.float16
BF16 = mybir.dt.bfloat16
FP8 = mybir.dt.float8e4


def build_nc(NCc=NC_FULL, D=D_FULL, S=S_FULL, psum_w=1024,
             mm_n=512, psum_bufs=4, eb_engine="vector", warmup=32,
             xn_early=3, x_chunks=2):
    """Build the single-core Bass program (same program runs SPMD on 8 cores)."""
    nc = bacc.Bacc("TRN2", target_bir_lowering=False, debug=False)

    xT = nc.dram_tensor("xT", [D, NCc], FP8, kind="ExternalInput").ap()
    xn = nc.dram_tensor("xn", [NCc, D], F16, kind="ExternalInput").ap()
    w = nc.dram_tensor("w", [D, S], FP8, kind="ExternalInput").ap()
    bv = nc.dram_tensor("bias", [S], BF16, kind="ExternalInput").ap()
    out = nc.dram_tensor("out", [NCc, S], BF16, kind="ExternalOutput").ap()

    KT = D // P            # k strips (contraction)
    KP = KT // 2           # DoubleRow k-pairs per psum group
    NB = NCc // P          # 128-row output blocks
    NS = min(mm_n, S)      # matmul moving free dim (<= 512 for one PSUM bank)
    S2 = min(psum_w, S)    # psum tile width
    SH = S // S2           # psum tiles per row block
    neg_half_ln_s = float(-0.5 * np.log(S))
    DR = mybir.MatmulPerfMode.DoubleRow

    with tile.TileContext(nc) as tc, ExitStack() as ctx:
        singles = ctx.enter_context(tc.tile_pool(name="singles", bufs=1))
        w_sb = singles.tile([P, KT, S], FP8)
        x_sb = singles.tile([P, KT, NCc], FP8)
        b_bc = singles.tile([P, S], BF16)
        eb = singles.tile([P, S], BF16)
        bias_tiles = [
            singles.tile([P, 1], F32, tag=f"bias{nb}", name=f"bias{nb}")
            for nb in range(NB)
        ]

        xr = xT.rearrange("(k p) n -> p k n", p=P)
        wr = w.rearrange("(k p) s -> p k s", p=P)

        # r-path: natural-layout x blocks (fp16) -> per-partition exp bias.
        xn_pool = ctx.enter_context(tc.tile_pool(name="xnp", bufs=4))
        sq_pool = ctx.enter_context(tc.tile_pool(name="sqp", bufs=2))
        r_pool = ctx.enter_context(tc.tile_pool(name="rp", bufs=4))
        xn_tiles = {}

        def load_xn(nb, eng=None, bufs=None):
            eng = eng or nc.scalar
            kw = {"bufs": bufs} if bufs else {}
            tag = f"xne{nb}" if bufs else "xns"
            xt = xn_pool.tile([P, D], F16, tag=tag, name=f"xn{nb}", **kw)
            eng.dma_start(xt, xn[nb * P:(nb + 1) * P, :])
            xn_tiles[nb] = xt

        def r_bias(nb):
            xt = xn_tiles[nb]
            sq = sq_pool.tile([P, D], F16)
            nc.vector.tensor_mul(sq, xt, xt)
            r_raw = r_pool.tile([P, 1], F32)
            nc.vector.tensor_reduce(
                r_raw, sq, axis=mybir.AxisListType.X, op=mybir.AluOpType.add)
            nc.vector.tensor_scalar(
                out=bias_tiles[nb], in0=r_raw,
                scalar1=-0.5, scalar2=neg_half_ln_s,
                op0=mybir.AluOpType.mult, op1=mybir.AluOpType.add)

        # sync ring: x fp8 strips, chunked along n so the first row blocks'
        # matmuls can start after ~1/x_chunks of the transfer; then outputs.
        XC = NCc // x_chunks
        for c in range(x_chunks):
            nc.sync.dma_start(
                x_sb[:, :, c * XC:(c + 1) * XC], xr[:, :, c * XC:(c + 1) * XC])

        # scalar ring: W half 0 (the h-outer sweep only needs the first half
        # for the very first psum group), xn0 (bias0 gates the first ACT and
        # thus PSUM recycling), W half 1, then b + more early xn.
        nc.scalar.dma_start(w_sb[:, :, 0:S // 2], wr[:, :, 0:S // 2])
        load_xn(0, bufs=1)
        nc.scalar.dma_start(w_sb[:, :, S // 2:S], wr[:, :, S // 2:S])
        bv_bcast = bass.AP(tensor=bv.tensor, offset=bv.offset,
                           ap=[[0, P]] + list(bv.ap))
        nc.scalar.dma_start(b_bc, bv_bcast)
        nc.scalar.activation(eb, b_bc, func=mybir.ActivationFunctionType.Exp)
        for nb in range(1, min(xn_early, NB)):
            load_xn(nb, bufs=1)

        for nb in range(min(xn_early + 2, NB)):
            if nb >= xn_early:
                load_xn(nb)
            if nb < min(xn_early, NB):
                r_bias(nb)

        psum_pool = ctx.enter_context(
            tc.tile_pool(name="psum", bufs=psum_bufs, space="PSUM"))
        tmp_pool = ctx.enter_context(tc.tile_pool(name="tmp", bufs=3))
        out_pool = ctx.enter_context(tc.tile_pool(name="osb", bufs=4))

        if warmup:
            # keep the PE busy (and HAM-warm) while the operand strips
            # stream in; results are discarded
            dummy_x = singles.tile([P, P], BF16)
            dummy_w = singles.tile([P, NS], BF16)
            nc.vector.memset(dummy_x, 0.0)
            nc.vector.memset(dummy_w, 0.0)
            for i in range(warmup):
                wps = psum_pool.tile([P, S2], F32, tag="ps", name=f"warm{i}")
                nc.tensor.matmul(wps[:, 0:NS], lhsT=dummy_x, rhs=dummy_w,
                                 start=True, stop=True)

        for nb in range(NB):
            nxt = nb + xn_early + 2
            if nxt < NB:
                load_xn(nxt)
            # k-pair outer within each psum group: one stationary x block
            # feeds S2/NS matmuls before the PE reloads weights.
            for h in range(SH):
                ps = psum_pool.tile([P, S2], F32, tag="ps", name=f"ps{nb}_{h}")
                for kp in range(KP):
                    lhsT = x_sb[:, 2 * kp:2 * kp + 2, nb * P:(nb + 1) * P]
                    for c in range(S2 // NS):
                        col0 = h * S2 + c * NS
                        nc.tensor.matmul(
                            ps[:, c * NS:(c + 1) * NS],
                            lhsT=lhsT,
                            rhs=w_sb[:, 2 * kp:2 * kp + 2, col0:col0 + NS],
                            start=(kp == 0),
                            stop=(kp == KP - 1),
                            perf_mode=DR,
                        )
                tmp = tmp_pool.tile([P, S2], BF16)
                nc.scalar.activation(
                    tmp, ps,
                    func=mybir.ActivationFunctionType.Exp,
                    bias=bias_tiles[nb],
                    scale=float(1.0 / W_SCALE),
                )
                hsl = slice(h * S2, (h + 1) * S2)
                o_sb = out_pool.tile([P, S2], BF16)
                eng = nc.gpsimd if (eb_engine == "gpsimd" and h % 2 == 0) \
                    else nc.vector
                eng.tensor_mul(o_sb, tmp, eb[:, hsl])
                nc.sync.dma_start(out[nb * P:(nb + 1) * P, hsl], o_sb)
            if nb + 3 < NB:
                r_bias(nb + 3)

    nc.compile()
    return nc


_NC_CACHE = {}


def _get_nc(**kwargs):
    key = tuple(sorted(kwargs.items()))
    if key not in _NC_CACHE:
        _NC_CACHE[key] = build_nc(**kwargs)
    return _NC_CACHE[key]


def make_in_maps(x, W, b):
    import ml_dtypes
    fp8 = ml_dtypes.float8_e4m3
    bf16 = ml_dtypes.bfloat16
    wT = np.ascontiguousarray((W.T * W_SCALE).astype(fp8))
    b = np.ascontiguousarray(b.astype(bf16))
    in_maps = []
    for i in range(N_CORES):
        xs = x[i * NC_FULL:(i + 1) * NC_FULL]
        in_maps.append({
            "xT": np.ascontiguousarray(xs.T.astype(fp8)),
            "xn": np.ascontiguousarray(xs.astype(np.float16)),
            "w": wT,
            "bias": b,
        })
    return in_maps


def run_hw(x, W, b, trace=False, **build_kwargs):
    """Run on 8 NeuronCores; returns (out [N, S] f32, BassKernelResults)."""
    from concourse.bass_utils import run_bass_kernel_spmd
    from concourse.bass_interp import get_hw_module

    nc = _get_nc(**build_kwargs)
    in_maps = make_in_maps(x, W, b)
    old_m = nc.m
    nc.m = get_hw_module(nc.m)
    try:
        res = run_bass_kernel_spmd(
            nc, in_maps, core_ids=list(range(N_CORES)), trace=trace)
    finally:
        nc.m = old_m
    out = np.concatenate(
        [res.results[i]["out"] for i in range(N_CORES)], axis=0)
    return out.astype(np.float32), res


def kernel(x, W, b):
    out, _ = run_hw(x, W, b, trace=False)
    return out


# revision 16
# speedup vs baseline: 1.0518x; 1.0518x over previous
"""Trainium2 Bass kernel for the Performer-style random-feature map:

    out[n, s] = exp(-||x_n||^2 / 2) * S^{-1/2} * exp((x @ W.T)[n, s] + b[s])
              = exp((x @ W.T)[n, s] - 0.5*||x_n||^2 - 0.5*ln(S)) * exp(b[s])

Sharding: data-parallel over the N (row) axis across 8 NeuronCores; W and b
replicated.  Each core computes a [2048, 2048] output block.  Pure SPMD, no
collectives.

Per-core structure (sizes hardcoded for N=16384, D=1024, S=2048):
  - x^T and W^T live in SBUF as fp8(e4m3) [128, 8, *] k-strip stacks; the
    matmul contracts 256 elements per instruction via DoubleRow perf mode
    (2 fp8 weights per PE cell -> ~1.5x bf16 throughput).  W is pre-scaled
    by 16 on the host so its values sit in the fp8 normal range; the 1/16
    is folded into the ACT exp scale.  The k loop is outside the column
    loop so one stationary x block serves 4 matmuls (amortizes LDWEIGHTS).
  - natural-layout x rows stream in as fp16 per 128-row block; one DVE
    tensor_tensor_reduce computes bias_n = -0.5*||x_n||^2 - 0.5*ln(S).
  - per row block: 2x [128, 1024] PSUM groups -> ACT exp(psum/16 + bias_n)
    in bf16 -> DVE bf16 multiply by exp(b) broadcast (2x packed mode) ->
    DMA out in bf16 (host upcasts to f32; the tolerance and the actual
    output range make this exact here).
"""

import sys
from contextlib import ExitStack

if "/opt/trn_rl_repo" not in sys.path:
    sys.path.insert(0, "/opt/trn_rl_repo")

import numpy as np

import concourse.bacc as bacc
import concourse.bass as bass
import concourse.tile as tile
from concourse import mybir

P = 128          # SBUF partitions
N_FULL = 16384   # total rows
D_FULL = 1024    # contraction dim
S_FULL = 2048    # output features
N_CORES = 8
NC_FULL = N_FULL // N_CORES  # rows per core

W_SCALE = 16.0   # host-side W multiplier (keeps fp8 W in the normal range)

F32 = mybir.dt.float32
F16 = mybir.dt.float16
BF16 = mybir.dt.bfloat16
FP8 = mybir.dt.float8e4


def build_nc(NCc=NC_FULL, D=D_FULL, S=S_FULL, psum_w=1024,
             mm_n=512, psum_bufs=4, eb_engine="vector", warmup=28,
             xn_early=3, x_chunks=4):
    """Build the single-core Bass program (same program runs SPMD on 8 cores)."""
    nc = bacc.Bacc("TRN2", target_bir_lowering=False, debug=False)

    xT = nc.dram_tensor("xT", [D, NCc], FP8, kind="ExternalInput").ap()
    xn = nc.dram_tensor("xn", [NCc, D], F16, kind="ExternalInput").ap()
    w = nc.dram_tensor("w", [D, S], FP8, kind="ExternalInput").ap()
    bv = nc.dram_tensor("bias", [S], BF16, kind="ExternalInput").ap()
    out = nc.dram_tensor("out", [NCc, S], BF16, kind="ExternalOutput").ap()

    KT = D // P            # k strips (contraction)
    KP = KT // 2           # DoubleRow k-pairs per psum group
    NB = NCc // P          # 128-row output blocks
    NS = min(mm_n, S)      # matmul moving free dim (<= 512 for one PSUM bank)
    S2 = min(psum_w, S)    # psum tile width
    SH = S // S2           # psum tiles per row block
    neg_half_ln_s = float(-0.5 * np.log(S))
    DR = mybir.MatmulPerfMode.DoubleRow

    with tile.TileContext(nc) as tc, ExitStack() as ctx:
        singles = ctx.enter_context(tc.tile_pool(name="singles", bufs=1))
        w_sb = singles.tile([P, KT, S], FP8)
        x_sb = singles.tile([P, KT, NCc], FP8)
        eb = singles.tile([P, S], BF16)
        bias_tiles = [
            singles.tile([P, 1], F32, tag=f"bias{nb}", name=f"bias{nb}")
            for nb in range(NB)
        ]

        xr = xT.rearrange("(k p) n -> p k n", p=P)
        wr = w.rearrange("(k p) s -> p k s", p=P)

        # r-path: natural-layout x blocks (fp16) -> per-partition exp bias.
        xn_pool = ctx.enter_context(tc.tile_pool(name="xnp", bufs=4))
        sq_pool = ctx.enter_context(tc.tile_pool(name="sqp", bufs=2))
        r_pool = ctx.enter_context(tc.tile_pool(name="rp", bufs=4))
        xn_tiles = {}

        def load_xn(nb, eng=None, bufs=None):
            eng = eng or nc.scalar
            kw = {"bufs": bufs} if bufs else {}
            tag = f"xne{nb}" if bufs else "xns"
            xt = xn_pool.tile([P, D], F16, tag=tag, name=f"xn{nb}", **kw)
            eng.dma_start(xt, xn[nb * P:(nb + 1) * P, :])
            xn_tiles[nb] = xt

        def r_bias(nb):
            xt = xn_tiles[nb]
            sq = sq_pool.tile([P, D], F16)
            nc.vector.tensor_mul(sq, xt, xt)
            r_raw = r_pool.tile([P, 1], F32)
            nc.vector.tensor_reduce(
                r_raw, sq, axis=mybir.AxisListType.X, op=mybir.AluOpType.add)
            nc.vector.tensor_scalar(
                out=bias_tiles[nb], in0=r_raw,
                scalar1=-0.5, scalar2=neg_half_ln_s,
                op0=mybir.AluOpType.mult, op1=mybir.AluOpType.add)

        # sync ring: x fp8 strips, chunked along n so the first row blocks'
        # matmuls can start after ~1/x_chunks of the transfer; then outputs.
        # front loads split across both HWDGE rings so W and the first x
        # half stream in parallel:
        #   sync ring:   x blocks 0..7 (1MiB), later the output tiles
        #   scalar ring: W halves, xn0, x blocks 8..15, b, more xn
        half = NCc // 2
        nc.sync.dma_start(x_sb[:, :, 0:half], xr[:, :, 0:half])
        nc.scalar.dma_start(w_sb[:, :, 0:S // 2], wr[:, :, 0:S // 2])
        nc.scalar.dma_start(w_sb[:, :, S // 2:S], wr[:, :, S // 2:S])
        load_xn(0, bufs=1)
        nc.scalar.dma_start(x_sb[:, :, half:NCc], xr[:, :, half:NCc])
        # eb = exp(b) is folded on the host; broadcast it to all partitions
        bv_bcast = bass.AP(tensor=bv.tensor, offset=bv.offset,
                           ap=[[0, P]] + list(bv.ap))
        nc.sync.dma_start(eb, bv_bcast)
        for nb in range(1, min(xn_early, NB)):
            load_xn(nb, bufs=1)

        for nb in range(min(xn_early + 2, NB)):
            if nb >= xn_early:
                load_xn(nb)
            if nb < min(xn_early, NB):
                r_bias(nb)

        psum_pool = ctx.enter_context(
            tc.tile_pool(name="psum", bufs=psum_bufs, space="PSUM"))
        tmp_pool = ctx.enter_context(tc.tile_pool(name="tmp", bufs=3))
        out_pool = ctx.enter_context(tc.tile_pool(name="osb", bufs=4))

        if warmup:
            # keep the PE busy (and HAM-warm) while the operand strips
            # stream in; results are discarded
            dummy_x = singles.tile([P, P], BF16)
            dummy_w = singles.tile([P, NS], BF16)
            nc.vector.memset(dummy_x, 0.0)
            nc.vector.memset(dummy_w, 0.0)
            for i in range(warmup):
                wps = psum_pool.tile([P, S2], F32, tag="ps", name=f"warm{i}")
                nc.tensor.matmul(wps[:, 0:NS], lhsT=dummy_x, rhs=dummy_w,
                                 start=True, stop=True)

        for nb in range(NB):
            nxt = nb + xn_early + 2
            if nxt < NB:
                load_xn(nxt)
            # k-pair outer within each psum group: one stationary x block
            # feeds S2/NS matmuls before the PE reloads weights.
            for h in range(SH):
                ps = psum_pool.tile([P, S2], F32, tag="ps", name=f"ps{nb}_{h}")
                for kp in range(KP):
                    lhsT = x_sb[:, 2 * kp:2 * kp + 2, nb * P:(nb + 1) * P]
                    for c in range(S2 // NS):
                        col0 = h * S2 + c * NS
                        nc.tensor.matmul(
                            ps[:, c * NS:(c + 1) * NS],
                            lhsT=lhsT,
                            rhs=w_sb[:, 2 * kp:2 * kp + 2, col0:col0 + NS],
                            start=(kp == 0),
                            stop=(kp == KP - 1),
                            perf_mode=DR,
                        )
                tmp = tmp_pool.tile([P, S2], BF16)
                nc.scalar.activation(
                    tmp, ps,
                    func=mybir.ActivationFunctionType.Exp,
                    bias=bias_tiles[nb],
                    scale=float(1.0 / W_SCALE),
                )
                hsl = slice(h * S2, (h + 1) * S2)
                o_sb = out_pool.tile([P, S2], BF16)
                eng = nc.gpsimd if (eb_engine == "gpsimd" and h % 2 == 0) \
                    else nc.vector
                eng.tensor_mul(o_sb, tmp, eb[:, hsl])
                nc.sync.dma_start(out[nb * P:(nb + 1) * P, hsl], o_sb)
            if nb + 3 < NB:
                r_bias(nb + 3)

    nc.compile()
    return nc


_NC_CACHE = {}


def _get_nc(**kwargs):
    key = tuple(sorted(kwargs.items()))
    if key not in _NC_CACHE:
        _NC_CACHE[key] = build_nc(**kwargs)
    return _NC_CACHE[key]


def make_in_maps(x, W, b):
    import ml_dtypes
    fp8 = ml_dtypes.float8_e4m3
    bf16 = ml_dtypes.bfloat16
    wT = np.ascontiguousarray((W.T * W_SCALE).astype(fp8))
    b = np.ascontiguousarray(np.exp(b.astype(np.float64)).astype(bf16))
    in_maps = []
    for i in range(N_CORES):
        xs = x[i * NC_FULL:(i + 1) * NC_FULL]
        in_maps.append({
            "xT": np.ascontiguousarray(xs.T.astype(fp8)),
            "xn": np.ascontiguousarray(xs.astype(np.float16)),
            "w": wT,
            "bias": b,
        })
    return in_maps


def run_hw(x, W, b, trace=False, **build_kwargs):
    """Run on 8 NeuronCores; returns (out [N, S] f32, BassKernelResults)."""
    from concourse.bass_utils import run_bass_kernel_spmd
    from concourse.bass_interp import get_hw_module

    nc = _get_nc(**build_kwargs)
    in_maps = make_in_maps(x, W, b)
    old_m = nc.m
    nc.m = get_hw_module(nc.m)
    try:
        res = run_bass_kernel_spmd(
            nc, in_maps, core_ids=list(range(N_CORES)), trace=trace)
    finally:
        nc.m = old_m
    out = np.concatenate(
        [res.results[i]["out"] for i in range(N_CORES)], axis=0)
    return out.astype(np.float32), res


def kernel(x, W, b):
    out, _ = run_hw(x, W, b, trace=False)
    return out


# revision 34
# speedup vs baseline: 1.0632x; 1.0108x over previous
"""Trainium2 Bass kernel for the Performer-style random-feature map:

    out[n, s] = exp(-||x_n||^2 / 2) * S^{-1/2} * exp((x @ W.T)[n, s] + b[s])
              = exp((x @ W.T)[n, s] - 0.5*||x_n||^2 - 0.5*ln(S)) * exp(b[s])

Sharding: data-parallel over the N (row) axis across 8 NeuronCores; W and b
replicated.  Each core computes a [2048, 2048] output block.  Pure SPMD, no
collectives.

Per-core structure (sizes hardcoded for N=16384, D=1024, S=2048):
  - x^T and W^T live in SBUF as fp8(e4m3) [128, 8, *] k-strip stacks; the
    matmul contracts 256 elements per instruction via DoubleRow perf mode
    (2 fp8 weights per PE cell -> ~1.5x bf16 throughput).  W is pre-scaled
    by 16 on the host so its values sit in the fp8 normal range; the 1/16
    is folded into the ACT exp scale.
  - the column dimension is swept h-outer: all 16 row blocks' first
    [128, 1024] psum groups run before any second group, so only half of W
    gates the start and the front HBM bandwidth goes to the x strips.
  - within a group the k loop is outer, so one stationary x block feeds
    S2/NS matmuls before the PE reloads weights (amortizes LDWEIGHTS).
  - natural-layout x rows stream in as fp16 per 128-row block; DVE computes
    bias_n = -0.5*||x_n||^2 - 0.5*ln(S) as a per-partition scalar.
  - per psum group: 8 accumulating DoubleRow matmuls -> ACT exp(psum/16 +
    bias_n) in bf16 -> DVE bf16 multiply by exp(b) (host-folded constant,
    broadcast once) -> DMA out in bf16 (host upcasts to f32; the tolerance
    and the actual output range make the cast exact here).
  - the very last psum group drains in 512-wide pieces on the otherwise
    idle scalar DMA ring to shorten the final exp->mul->store chain.
"""

import sys
from contextlib import ExitStack

if "/opt/trn_rl_repo" not in sys.path:
    sys.path.insert(0, "/opt/trn_rl_repo")

import numpy as np

import concourse.bacc as bacc
import concourse.bass as bass
import concourse.tile as tile
from concourse import mybir

P = 128          # SBUF partitions
N_FULL = 16384   # total rows
D_FULL = 1024    # contraction dim
S_FULL = 2048    # output features
N_CORES = 8
NC_FULL = N_FULL // N_CORES  # rows per core

W_SCALE = 16.0   # host-side W multiplier (keeps fp8 W in the normal range)

F32 = mybir.dt.float32
F16 = mybir.dt.float16
BF16 = mybir.dt.bfloat16
FP8 = mybir.dt.float8e4


def build_nc(NCc=NC_FULL, D=D_FULL, S=S_FULL, psum_w=1024,
             mm_n=512, psum_bufs=4, eb_engine="vector", warmup=28,
             xn_early=3):
    """Build the single-core Bass program (same program runs SPMD on 8 cores)."""
    nc = bacc.Bacc("TRN2", target_bir_lowering=False, debug=False)

    xT = nc.dram_tensor("xT", [D, NCc], FP8, kind="ExternalInput").ap()
    xn = nc.dram_tensor("xn", [NCc, D], F16, kind="ExternalInput").ap()
    w = nc.dram_tensor("w", [D, S], FP8, kind="ExternalInput").ap()
    bv = nc.dram_tensor("bias", [S], BF16, kind="ExternalInput").ap()
    out = nc.dram_tensor("out", [NCc, S], BF16, kind="ExternalOutput").ap()

    KT = D // P            # k strips (contraction)
    KP = KT // 2           # DoubleRow k-pairs per psum group
    NB = NCc // P          # 128-row output blocks
    NS = min(mm_n, S)      # matmul moving free dim (<= 512 for one PSUM bank)
    S2 = min(psum_w, S)    # psum tile width
    SH = S // S2           # psum tiles per row block
    neg_half_ln_s = float(-0.5 * np.log(S))
    DR = mybir.MatmulPerfMode.DoubleRow

    with tile.TileContext(nc) as tc, ExitStack() as ctx:
        singles = ctx.enter_context(tc.tile_pool(name="singles", bufs=1))
        w_sb = singles.tile([P, KT, S], FP8)
        x_sb = singles.tile([P, KT, NCc], FP8)
        eb = singles.tile([P, S], BF16)
        bias_tiles = [
            singles.tile([P, 1], F32, tag=f"bias{nb}", name=f"bias{nb}")
            for nb in range(NB)
        ]

        xr = xT.rearrange("(k p) n -> p k n", p=P)
        wr = w.rearrange("(k p) s -> p k s", p=P)

        # r-path: natural-layout x blocks (fp16) -> per-partition exp bias.
        xn_pool = ctx.enter_context(tc.tile_pool(name="xnp", bufs=4))
        sq_pool = ctx.enter_context(tc.tile_pool(name="sqp", bufs=2))
        r_pool = ctx.enter_context(tc.tile_pool(name="rp", bufs=4))
        xn_tiles = {}

        def load_xn(nb, bufs=None):
            kw = {"bufs": bufs} if bufs else {}
            tag = f"xne{nb}" if bufs else "xns"
            xt = xn_pool.tile([P, D], F16, tag=tag, name=f"xn{nb}", **kw)
            nc.scalar.dma_start(xt, xn[nb * P:(nb + 1) * P, :])
            xn_tiles[nb] = xt

        def r_bias(nb):
            xt = xn_tiles[nb]
            sq = sq_pool.tile([P, D], F16)
            nc.vector.tensor_mul(sq, xt, xt)
            r_raw = r_pool.tile([P, 1], F32)
            nc.vector.tensor_reduce(
                r_raw, sq, axis=mybir.AxisListType.X, op=mybir.AluOpType.add)
            nc.vector.tensor_scalar(
                out=bias_tiles[nb], in0=r_raw,
                scalar1=-0.5, scalar2=neg_half_ln_s,
                op0=mybir.AluOpType.mult, op1=mybir.AluOpType.add)

        # front loads split across both HWDGE rings, ordered by when the
        # banded sweep needs them:
        #   sync ring:   x blocks 0..3, W half 1, x blocks 4..7, eb, outs
        #   scalar ring: W half 0, xn0..2, x blocks 8..15, jit xn
        q1, q2 = NCc // 4, NCc // 2
        nc.sync.dma_start(x_sb[:, :, 0:q1], xr[:, :, 0:q1])
        nc.scalar.dma_start(w_sb[:, :, 0:S // 2], wr[:, :, 0:S // 2])
        nc.sync.dma_start(w_sb[:, :, S // 2:S], wr[:, :, S // 2:S])
        load_xn(0, bufs=1)
        nc.sync.dma_start(x_sb[:, :, q1:q2], xr[:, :, q1:q2])
        for nb in range(1, min(xn_early, NB)):
            load_xn(nb, bufs=1)
        # eb = exp(b) is folded on the host; broadcast it to all partitions
        bv_bcast = bass.AP(tensor=bv.tensor, offset=bv.offset,
                           ap=[[0, P]] + list(bv.ap))
        nc.sync.dma_start(eb, bv_bcast)
        nc.scalar.dma_start(x_sb[:, :, q2:NCc], xr[:, :, q2:NCc])

        for nb in range(min(xn_early, NB)):
            r_bias(nb)

        psum_pool = ctx.enter_context(
            tc.tile_pool(name="psum", bufs=psum_bufs, space="PSUM"))
        tmp_pool = ctx.enter_context(tc.tile_pool(name="tmp", bufs=4))
        out_pool = ctx.enter_context(tc.tile_pool(name="osb", bufs=6))

        if warmup:
            # keep the PE busy (and HAM-warm) while the operand strips
            # stream in; results are discarded
            dummy_x = singles.tile([P, P], BF16)
            dummy_w = singles.tile([P, NS], BF16)
            nc.vector.memset(dummy_x, 0.0)
            nc.vector.memset(dummy_w, 0.0)
            for i in range(warmup):
                wps = psum_pool.tile([P, S2], F32, tag="ps", name=f"warm{i}")
                nc.tensor.matmul(wps[:, 0:NS], lhsT=dummy_x, rhs=dummy_w,
                                 start=True, stop=True)

        # xn loads / bias computations are issued one per group below;
        # r_bias goes ahead of the group's multiply in the DVE FIFO so a
        # multiply blocked on ACT can't delay later bias computations.
        sched = {"xn": xn_early, "rb": xn_early}

        def schedule_rbias():
            if sched["xn"] < NB:
                load_xn(sched["xn"])
                sched["xn"] += 1
            if sched["rb"] < NB:
                r_bias(sched["rb"])
                sched["rb"] += 1

        def drain(ps, nb, h, last):
            # the final psum group drains in NS-wide pieces so the last
            # exp->mul->store chain is short
            npc = S2 // NS if last else 1
            pw = S2 // npc
            for pc in range(npc):
                tmp = tmp_pool.tile([P, pw], BF16, tag="tmp")
                nc.scalar.activation(
                    tmp, ps[:, pc * pw:(pc + 1) * pw],
                    func=mybir.ActivationFunctionType.Exp,
                    bias=bias_tiles[nb],
                    scale=float(1.0 / W_SCALE),
                )
                hsl = slice(h * S2 + pc * pw, h * S2 + (pc + 1) * pw)
                o_sb = out_pool.tile([P, pw], BF16, tag="osb")
                nc.vector.tensor_mul(o_sb, tmp, eb[:, hsl])
                oeng = nc.scalar if last else nc.sync
                oeng.dma_start(out[nb * P:(nb + 1) * P, hsl], o_sb)

        # band 0 (row blocks 0..3) sweeps h-banded: the four h=0 psum
        # groups only touch W half 0, giving the W half-1 DMA headroom.
        BAND = 4
        for h in range(SH):
            for nb in range(BAND):
                if h == 0:
                    schedule_rbias()
                ps = psum_pool.tile([P, S2], F32, tag="ps", name=f"ps{nb}_{h}")
                for kp in range(KP):
                    lhsT = x_sb[:, 2 * kp:2 * kp + 2, nb * P:(nb + 1) * P]
                    for c in range(S2 // NS):
                        col0 = h * S2 + c * NS
                        nc.tensor.matmul(
                            ps[:, c * NS:(c + 1) * NS],
                            lhsT=lhsT,
                            rhs=w_sb[:, 2 * kp:2 * kp + 2, col0:col0 + NS],
                            start=(kp == 0),
                            stop=(kp == KP - 1),
                            perf_mode=DR,
                        )
                drain(ps, nb, h, False)

        # remaining row blocks: both psum groups interleaved in one kp
        # sweep so a stationary x block feeds 2*S2/NS matmuls (W is fully
        # resident by now).  The very last group runs its column banks
        # back-to-back so its first ACT piece overlaps the closing matmuls.
        for nb in range(BAND, NB):
            schedule_rbias()
            last_nb = nb == NB - 1
            pss = [psum_pool.tile([P, S2], F32, tag="ps", name=f"ps{nb}_{h}")
                   for h in range(SH)]
            if not last_nb:
                for kp in range(KP):
                    lhsT = x_sb[:, 2 * kp:2 * kp + 2, nb * P:(nb + 1) * P]
                    for h in range(SH):
                        for c in range(S2 // NS):
                            col0 = h * S2 + c * NS
                            nc.tensor.matmul(
                                pss[h][:, c * NS:(c + 1) * NS],
                                lhsT=lhsT,
                                rhs=w_sb[:, 2 * kp:2 * kp + 2, col0:col0 + NS],
                                start=(kp == 0),
                                stop=(kp == KP - 1),
                                perf_mode=DR,
                            )
                for h in range(SH):
                    drain(pss[h], nb, h, False)
            else:
                for h in range(SH):
                    for c in range(S2 // NS):
                        col0 = h * S2 + c * NS
                        for kp in range(KP):
                            nc.tensor.matmul(
                                pss[h][:, c * NS:(c + 1) * NS],
                                lhsT=x_sb[:, 2 * kp:2 * kp + 2,
                                          nb * P:(nb + 1) * P],
                                rhs=w_sb[:, 2 * kp:2 * kp + 2, col0:col0 + NS],
                                start=(kp == 0),
                                stop=(kp == KP - 1),
                                perf_mode=DR,
                            )
                    drain(pss[h], nb, h, h == SH - 1)

    nc.compile()
    return nc


_NC_CACHE = {}


def _get_nc(**kwargs):
    key = tuple(sorted(kwargs.items()))
    if key not in _NC_CACHE:
        _NC_CACHE[key] = build_nc(**kwargs)
    return _NC_CACHE[key]


def make_in_maps(x, W, b):
    import ml_dtypes
    fp8 = ml_dtypes.float8_e4m3
    bf16 = ml_dtypes.bfloat16
    wT = np.ascontiguousarray((W.T * W_SCALE).astype(fp8))
    b = np.ascontiguousarray(np.exp(b.astype(np.float64)).astype(bf16))
    in_maps = []
    for i in range(N_CORES):
        xs = x[i * NC_FULL:(i + 1) * NC_FULL]
        in_maps.append({
            "xT": np.ascontiguousarray(xs.T.astype(fp8)),
            "xn": np.ascontiguousarray(xs.astype(np.float16)),
            "w": wT,
            "bias": b,
        })
    return in_maps


def run_hw(x, W, b, trace=False, **build_kwargs):
    """Run on 8 NeuronCores; returns (out [N, S] f32, BassKernelResults)."""
    from concourse.bass_utils import run_bass_kernel_spmd
    from concourse.bass_interp import get_hw_module

    nc = _get_nc(**build_kwargs)
    in_maps = make_in_maps(x, W, b)
    old_m = nc.m
    nc.m = get_hw_module(nc.m)
    try:
        res = run_bass_kernel_spmd(
            nc, in_maps, core_ids=list(range(N_CORES)), trace=trace)
    finally:
        nc.m = old_m
    out = np.concatenate(
        [res.results[i]["out"] for i in range(N_CORES)], axis=0)
    return out.astype(np.float32), res


def kernel(x, W, b):
    out, _ = run_hw(x, W, b, trace=False)
    return out


# revision 36
# speedup vs baseline: 1.2427x; 1.1689x over previous
"""Trainium2 Bass kernel for the Performer-style random-feature map:

    out[n, s] = exp(-||x_n||^2 / 2) * S^{-1/2} * exp((x @ W.T)[n, s] + b[s])
              = exp((x @ W.T)[n, s] - 0.5*||x_n||^2 - 0.5*ln(S)) * exp(b[s])

Sharding: data-parallel over the N (row) axis across 8 NeuronCores; W and b
replicated.  Each core computes a [2048, 2048] output block.  Pure SPMD, no
collectives.

Per-core structure (sizes hardcoded for N=16384, D=1024, S=2048):
  - x^T and W^T live in SBUF as fp8(e4m3) [128, 8, *] k-strip stacks; the
    matmul contracts 256 elements per instruction via DoubleRow perf mode
    (2 fp8 weights per PE cell -> ~1.5x bf16 throughput).  W is pre-scaled
    by 16 on the host so its values sit in the fp8 normal range; the 1/16
    is folded into the ACT exp scale.
  - the column dimension is swept h-outer: all 16 row blocks' first
    [128, 1024] psum groups run before any second group, so only half of W
    gates the start and the front HBM bandwidth goes to the x strips.
  - within a group the k loop is outer, so one stationary x block feeds
    S2/NS matmuls before the PE reloads weights (amortizes LDWEIGHTS).
  - natural-layout x rows stream in as fp16 per 128-row block; DVE computes
    bias_n = -0.5*||x_n||^2 - 0.5*ln(S) as a per-partition scalar.
  - per psum group: 8 accumulating DoubleRow matmuls -> ACT exp(psum/16 +
    bias_n) in bf16 -> DVE bf16 multiply by exp(b) (host-folded constant,
    broadcast once) -> DMA out in bf16 (host upcasts to f32; the tolerance
    and the actual output range make the cast exact here).
  - the very last psum group drains in 512-wide pieces on the otherwise
    idle scalar DMA ring to shorten the final exp->mul->store chain.
"""

import sys
from contextlib import ExitStack

if "/opt/trn_rl_repo" not in sys.path:
    sys.path.insert(0, "/opt/trn_rl_repo")

import numpy as np

import concourse.bacc as bacc
import concourse.bass as bass
import concourse.tile as tile
from concourse import mybir

P = 128          # SBUF partitions
N_FULL = 16384   # total rows
D_FULL = 1024    # contraction dim
S_FULL = 2048    # output features
N_CORES = 8
NC_FULL = N_FULL // N_CORES  # rows per core

W_SCALE = 16.0   # host-side W multiplier (keeps fp8 W in the normal range)

F32 = mybir.dt.float32
F16 = mybir.dt.float16
BF16 = mybir.dt.bfloat16
FP8 = mybir.dt.float8e4


def build_nc(NCc=NC_FULL, D=D_FULL, S=S_FULL, psum_w=1024,
             mm_n=512, psum_bufs=4, eb_engine="vector", warmup=28,
             xn_early=3):
    """Build the single-core Bass program (same program runs SPMD on 8 cores)."""
    nc = bacc.Bacc("TRN2", target_bir_lowering=False, debug=False)

    xT = nc.dram_tensor("xT", [D, NCc], FP8, kind="ExternalInput").ap()
    xn = nc.dram_tensor("xn", [NCc, D], F16, kind="ExternalInput").ap()
    w = nc.dram_tensor("w", [D, S], FP8, kind="ExternalInput").ap()
    bv = nc.dram_tensor("bias", [S], BF16, kind="ExternalInput").ap()
    out = nc.dram_tensor("out", [NCc, S], BF16, kind="ExternalOutput").ap()

    KT = D // P            # k strips (contraction)
    KP = KT // 2           # DoubleRow k-pairs per psum group
    NB = NCc // P          # 128-row output blocks
    NS = min(mm_n, S)      # matmul moving free dim (<= 512 for one PSUM bank)
    S2 = min(psum_w, S)    # psum tile width
    SH = S // S2           # psum tiles per row block
    neg_half_ln_s = float(-0.5 * np.log(S))
    DR = mybir.MatmulPerfMode.DoubleRow

    with tile.TileContext(nc) as tc, ExitStack() as ctx:
        singles = ctx.enter_context(tc.tile_pool(name="singles", bufs=1))
        w_sb = singles.tile([P, KT, S], FP8)
        x_sb = singles.tile([P, KT, NCc], FP8)
        eb = singles.tile([P, S], BF16)
        bias_tiles = [
            singles.tile([P, 1], F32, tag=f"bias{nb}", name=f"bias{nb}")
            for nb in range(NB)
        ]

        xr = xT.rearrange("(k p) n -> p k n", p=P)
        wr = w.rearrange("(k p) s -> p k s", p=P)

        # r-path: natural-layout x blocks (fp16) -> per-partition exp bias.
        xn_pool = ctx.enter_context(tc.tile_pool(name="xnp", bufs=4))
        sq_pool = ctx.enter_context(tc.tile_pool(name="sqp", bufs=2))
        r_pool = ctx.enter_context(tc.tile_pool(name="rp", bufs=4))
        xn_tiles = {}

        def load_xn(nb, bufs=None):
            kw = {"bufs": bufs} if bufs else {}
            tag = f"xne{nb}" if bufs else "xns"
            xt = xn_pool.tile([P, D], F16, tag=tag, name=f"xn{nb}", **kw)
            nc.scalar.dma_start(xt, xn[nb * P:(nb + 1) * P, :])
            xn_tiles[nb] = xt

        def r_bias(nb):
            xt = xn_tiles[nb]
            sq = sq_pool.tile([P, D], F16)
            nc.vector.tensor_mul(sq, xt, xt)
            r_raw = r_pool.tile([P, 1], F32)
            nc.vector.tensor_reduce(
                r_raw, sq, axis=mybir.AxisListType.X, op=mybir.AluOpType.add)
            nc.vector.tensor_scalar(
                out=bias_tiles[nb], in0=r_raw,
                scalar1=-0.5, scalar2=neg_half_ln_s,
                op0=mybir.AluOpType.mult, op1=mybir.AluOpType.add)

        # front loads split across both HWDGE rings, ordered by when the
        # banded sweep needs them:
        #   sync ring:   x blocks 0..3, W half 1, x blocks 4..7, eb, outs
        #   scalar ring: W half 0, xn0..2, x blocks 8..15, jit xn
        q1, q2 = NCc // 4, NCc // 2
        nc.sync.dma_start(x_sb[:, :, 0:q1], xr[:, :, 0:q1])
        nc.scalar.dma_start(w_sb[:, :, 0:S // 2], wr[:, :, 0:S // 2])
        nc.sync.dma_start(w_sb[:, :, S // 2:S], wr[:, :, S // 2:S])
        load_xn(0, bufs=1)
        nc.sync.dma_start(x_sb[:, :, q1:q2], xr[:, :, q1:q2])
        for nb in range(1, min(xn_early, NB)):
            load_xn(nb, bufs=1)
        # eb = exp(b) is folded on the host; broadcast it to all partitions
        bv_bcast = bass.AP(tensor=bv.tensor, offset=bv.offset,
                           ap=[[0, P]] + list(bv.ap))
        nc.sync.dma_start(eb, bv_bcast)
        nc.scalar.dma_start(x_sb[:, :, q2:NCc], xr[:, :, q2:NCc])

        for nb in range(min(xn_early, NB)):
            r_bias(nb)

        psum_pool = ctx.enter_context(
            tc.tile_pool(name="psum", bufs=psum_bufs, space="PSUM"))
        tmp_pool = ctx.enter_context(tc.tile_pool(name="tmp", bufs=4))
        out_pool = ctx.enter_context(tc.tile_pool(name="osb", bufs=6))

        if warmup:
            # keep the PE busy (and HAM-warm) while the operand strips
            # stream in; results are discarded
            dummy_x = singles.tile([P, P], BF16)
            dummy_w = singles.tile([P, NS], BF16)
            nc.vector.memset(dummy_x, 0.0)
            nc.vector.memset(dummy_w, 0.0)
            for i in range(warmup):
                wps = psum_pool.tile([P, S2], F32, tag="ps", name=f"warm{i}")
                nc.tensor.matmul(wps[:, 0:NS], lhsT=dummy_x, rhs=dummy_w,
                                 start=True, stop=True)

        # xn loads / bias computations are issued one per group below;
        # r_bias goes ahead of the group's multiply in the DVE FIFO so a
        # multiply blocked on ACT can't delay later bias computations.
        sched = {"xn": xn_early, "rb": xn_early}

        def schedule_rbias():
            if sched["xn"] < NB:
                load_xn(sched["xn"])
                sched["xn"] += 1
            if sched["rb"] < NB:
                r_bias(sched["rb"])
                sched["rb"] += 1

        def drain(ps, nb, h, last):
            # the final psum group drains in NS-wide pieces so the last
            # exp->mul->store chain is short
            npc = S2 // NS if last else 1
            pw = S2 // npc
            for pc in range(npc):
                tmp = tmp_pool.tile([P, pw], BF16, tag="tmp")
                nc.scalar.activation(
                    tmp, ps[:, pc * pw:(pc + 1) * pw],
                    func=mybir.ActivationFunctionType.Exp,
                    bias=bias_tiles[nb],
                    scale=float(1.0 / W_SCALE),
                )
                hsl = slice(h * S2 + pc * pw, h * S2 + (pc + 1) * pw)
                o_sb = out_pool.tile([P, pw], BF16, tag="osb")
                nc.vector.tensor_mul(o_sb, tmp, eb[:, hsl])
                oeng = nc.scalar if last else nc.sync
                oeng.dma_start(out[nb * P:(nb + 1) * P, hsl], o_sb)

        # band 0 (row blocks 0..3) sweeps h-banded: the four h=0 psum
        # groups only touch W half 0, giving the W half-1 DMA headroom.
        BAND = 4
        for h in range(SH):
            for nb in range(BAND):
                if h == 0:
                    schedule_rbias()
                ps = psum_pool.tile([P, S2], F32, tag="ps", name=f"ps{nb}_{h}")
                for kp in range(KP):
                    lhsT = x_sb[:, 2 * kp:2 * kp + 2, nb * P:(nb + 1) * P]
                    for c in range(S2 // NS):
                        col0 = h * S2 + c * NS
                        nc.tensor.matmul(
                            ps[:, c * NS:(c + 1) * NS],
                            lhsT=lhsT,
                            rhs=w_sb[:, 2 * kp:2 * kp + 2, col0:col0 + NS],
                            start=(kp == 0),
                            stop=(kp == KP - 1),
                            perf_mode=DR,
                        )
                drain(ps, nb, h, False)

        # remaining row blocks: both psum groups interleaved in one kp
        # sweep so a stationary x block feeds 2*S2/NS matmuls (W is fully
        # resident by now).  The very last group runs its column banks
        # back-to-back so its first ACT piece overlaps the closing matmuls.
        for nb in range(BAND, NB):
            schedule_rbias()
            last_nb = nb == NB - 1
            pss = [psum_pool.tile([P, S2], F32, tag="ps", name=f"ps{nb}_{h}")
                   for h in range(SH)]
            if not last_nb:
                for kp in range(KP):
                    lhsT = x_sb[:, 2 * kp:2 * kp + 2, nb * P:(nb + 1) * P]
                    for h in range(SH):
                        for c in range(S2 // NS):
                            col0 = h * S2 + c * NS
                            nc.tensor.matmul(
                                pss[h][:, c * NS:(c + 1) * NS],
                                lhsT=lhsT,
                                rhs=w_sb[:, 2 * kp:2 * kp + 2, col0:col0 + NS],
                                start=(kp == 0),
                                stop=(kp == KP - 1),
                                perf_mode=DR,
                            )
                for h in range(SH):
                    drain(pss[h], nb, h, False)
            else:
                for h in range(SH):
                    for c in range(S2 // NS):
                        col0 = h * S2 + c * NS
                        for kp in range(KP):
                            nc.tensor.matmul(
                                pss[h][:, c * NS:(c + 1) * NS],
                                lhsT=x_sb[:, 2 * kp:2 * kp + 2,
                                          nb * P:(nb + 1) * P],
                                rhs=w_sb[:, 2 * kp:2 * kp + 2, col0:col0 + NS],
                                start=(kp == 0),
                                stop=(kp == KP - 1),
                                perf_mode=DR,
                            )
                    drain(pss[h], nb, h, h == SH - 1)

    nc.compile()
    return nc


_NC_CACHE = {}


def _get_nc(**kwargs):
    key = tuple(sorted(kwargs.items()))
    if key not in _NC_CACHE:
        _NC_CACHE[key] = build_nc(**kwargs)
    return _NC_CACHE[key]


def make_in_maps(x, W, b):
    import ml_dtypes
    fp8 = ml_dtypes.float8_e4m3
    bf16 = ml_dtypes.bfloat16
    wT = np.ascontiguousarray((W.T * W_SCALE).astype(fp8))
    b = np.ascontiguousarray(np.exp(b.astype(np.float64)).astype(bf16))
    in_maps = []
    for i in range(N_CORES):
        xs = x[i * NC_FULL:(i + 1) * NC_FULL]
        in_maps.append({
            "xT": np.ascontiguousarray(xs.T.astype(fp8)),
            "xn": np.ascontiguousarray(xs.astype(np.float16)),
            "w": wT,
            "bias": b,
        })
    return in_maps


def run_hw(x, W, b, trace=False, **build_kwargs):
    """Run on 8 NeuronCores; returns (out [N, S] f32, BassKernelResults)."""
    from concourse.bass_utils import run_bass_kernel_spmd
    from concourse.bass_interp import get_hw_module

    nc = _get_nc(**build_kwargs)
    in_maps = make_in_maps(x, W, b)
    old_m = nc.m
    nc.m = get_hw_module(nc.m)
    try:
        res = run_bass_kernel_spmd(
            nc, in_maps, core_ids=list(range(N_CORES)), trace=trace)
    finally:
        nc.m = old_m
    out = np.concatenate(
        [res.results[i]["out"] for i in range(N_CORES)], axis=0)
    return out.astype(np.float32), res


def kernel(x, W, b):
    out, _ = run_hw(x, W, b, trace=False)
    return out


# revision 40
# speedup vs baseline: 1.2513x; 1.0069x over previous
"""Trainium2 Bass kernel for the Performer-style random-feature map:

    out[n, s] = exp(-||x_n||^2 / 2) * S^{-1/2} * exp((x @ W.T)[n, s] + b[s])
              = exp((x @ W.T)[n, s] - 0.5*||x_n||^2 - 0.5*ln(S)) * exp(b[s])

Sharding: data-parallel over the N (row) axis across 8 NeuronCores; W and b
replicated.  Each core computes a [2048, 2048] output block.  Pure SPMD, no
collectives.

Per-core structure (sizes hardcoded for N=16384, D=1024, S=2048):
  - x^T and W^T live in SBUF as fp8(e4m3) [128, 8, *] k-strip stacks; the
    matmul contracts 256 elements per instruction via DoubleRow perf mode
    (2 fp8 weights per PE cell -> ~1.5x bf16 throughput).  W is pre-scaled
    by 16 on the host so its values sit in the fp8 normal range; the 1/16
    is folded into the ACT exp scale.
  - the column dimension is swept h-outer: all 16 row blocks' first
    [128, 1024] psum groups run before any second group, so only half of W
    gates the start and the front HBM bandwidth goes to the x strips.
  - within a group the k loop is outer, so one stationary x block feeds
    S2/NS matmuls before the PE reloads weights (amortizes LDWEIGHTS).
  - natural-layout x rows stream in as fp16 per 128-row block; DVE computes
    bias_n = -0.5*||x_n||^2 - 0.5*ln(S) as a per-partition scalar.
  - per psum group: 8 accumulating DoubleRow matmuls -> ACT exp(psum/16 +
    bias_n) in bf16 -> DVE bf16 multiply by exp(b) (host-folded constant,
    broadcast once) -> DMA out in bf16 (host upcasts to f32; the tolerance
    and the actual output range make the cast exact here).
  - the very last psum group drains in 512-wide pieces on the otherwise
    idle scalar DMA ring to shorten the final exp->mul->store chain.
"""

import sys
from contextlib import ExitStack

if "/opt/trn_rl_repo" not in sys.path:
    sys.path.insert(0, "/opt/trn_rl_repo")

import numpy as np

import concourse.bacc as bacc
import concourse.bass as bass
import concourse.tile as tile
from concourse import mybir

P = 128          # SBUF partitions
N_FULL = 16384   # total rows
D_FULL = 1024    # contraction dim
S_FULL = 2048    # output features
N_CORES = 8
NC_FULL = N_FULL // N_CORES  # rows per core

W_SCALE = 16.0   # host-side W multiplier (keeps fp8 W in the normal range)

F32 = mybir.dt.float32
F16 = mybir.dt.float16
BF16 = mybir.dt.bfloat16
FP8 = mybir.dt.float8e4


def build_nc(NCc=NC_FULL, D=D_FULL, S=S_FULL, psum_w=1024,
             mm_n=512, psum_bufs=4, eb_engine="vector", warmup=28,
             xn_early=3):
    """Build the single-core Bass program (same program runs SPMD on 8 cores)."""
    nc = bacc.Bacc("TRN2", target_bir_lowering=False, debug=False)

    xT = nc.dram_tensor("xT", [D, NCc], FP8, kind="ExternalInput").ap()
    xn = nc.dram_tensor("xn", [NCc, D], F16, kind="ExternalInput").ap()
    w = nc.dram_tensor("w", [D, S], FP8, kind="ExternalInput").ap()
    bv = nc.dram_tensor("bias", [S], BF16, kind="ExternalInput").ap()
    out = nc.dram_tensor("out", [NCc, S], BF16, kind="ExternalOutput").ap()

    KT = D // P            # k strips (contraction)
    KP = KT // 2           # DoubleRow k-pairs per psum group
    NB = NCc // P          # 128-row output blocks
    NS = min(mm_n, S)      # matmul moving free dim (<= 512 for one PSUM bank)
    S2 = min(psum_w, S)    # psum tile width
    SH = S // S2           # psum tiles per row block
    neg_half_ln_s = float(-0.5 * np.log(S))
    DR = mybir.MatmulPerfMode.DoubleRow

    with tile.TileContext(nc) as tc, ExitStack() as ctx:
        singles = ctx.enter_context(tc.tile_pool(name="singles", bufs=1))
        w_sb = singles.tile([P, KT, S], FP8)
        x_sb = singles.tile([P, KT, NCc], FP8)
        eb = singles.tile([P, S], BF16)
        bias_tiles = [
            singles.tile([P, 1], F32, tag=f"bias{nb}", name=f"bias{nb}")
            for nb in range(NB)
        ]

        xr = xT.rearrange("(k p) n -> p k n", p=P)
        wr = w.rearrange("(k p) s -> p k s", p=P)

        # r-path: natural-layout x blocks (fp16) -> per-partition exp bias.
        xn_pool = ctx.enter_context(tc.tile_pool(name="xnp", bufs=4))
        sq_pool = ctx.enter_context(tc.tile_pool(name="sqp", bufs=2))
        r_pool = ctx.enter_context(tc.tile_pool(name="rp", bufs=4))
        xn_tiles = {}

        def load_xn(nb, bufs=None):
            kw = {"bufs": bufs} if bufs else {}
            tag = f"xne{nb}" if bufs else "xns"
            xt = xn_pool.tile([P, D], F16, tag=tag, name=f"xn{nb}", **kw)
            nc.scalar.dma_start(xt, xn[nb * P:(nb + 1) * P, :])
            xn_tiles[nb] = xt

        def r_bias(nb):
            xt = xn_tiles[nb]
            sq = sq_pool.tile([P, D], F16)
            nc.vector.tensor_mul(sq, xt, xt)
            r_raw = r_pool.tile([P, 1], F32)
            nc.vector.tensor_reduce(
                r_raw, sq, axis=mybir.AxisListType.X, op=mybir.AluOpType.add)
            nc.vector.tensor_scalar(
                out=bias_tiles[nb], in0=r_raw,
                scalar1=-0.5, scalar2=neg_half_ln_s,
                op0=mybir.AluOpType.mult, op1=mybir.AluOpType.add)

        # front loads split across both HWDGE rings, ordered by when the
        # banded sweep needs them:
        #   sync ring:   x blocks 0..3, W half 1, x blocks 4..7, eb, outs
        #   scalar ring: W half 0, xn0..2, x blocks 8..15, jit xn
        q1, q2 = NCc // 4, NCc // 2
        nc.sync.dma_start(x_sb[:, :, 0:q1], xr[:, :, 0:q1])
        nc.scalar.dma_start(w_sb[:, :, 0:S // 2], wr[:, :, 0:S // 2])
        nc.sync.dma_start(w_sb[:, :, S // 2:S], wr[:, :, S // 2:S])
        load_xn(0, bufs=1)
        nc.sync.dma_start(x_sb[:, :, q1:q2], xr[:, :, q1:q2])
        for nb in range(1, min(xn_early, NB)):
            load_xn(nb, bufs=1)
        # eb = exp(b) is folded on the host; broadcast it to all partitions
        bv_bcast = bass.AP(tensor=bv.tensor, offset=bv.offset,
                           ap=[[0, P]] + list(bv.ap))
        nc.sync.dma_start(eb, bv_bcast)
        nc.scalar.dma_start(x_sb[:, :, q2:NCc], xr[:, :, q2:NCc])

        for nb in range(min(xn_early, NB)):
            r_bias(nb)

        psum_pool = ctx.enter_context(
            tc.tile_pool(name="psum", bufs=psum_bufs, space="PSUM"))
        tmp_pool = ctx.enter_context(tc.tile_pool(name="tmp", bufs=4))
        out_pool = ctx.enter_context(tc.tile_pool(name="osb", bufs=6))

        if warmup:
            # keep the PE busy (and HAM-warm) while the operand strips
            # stream in; results are discarded
            dummy_x = singles.tile([P, P], BF16)
            dummy_w = singles.tile([P, NS], BF16)
            nc.vector.memset(dummy_x, 0.0)
            nc.vector.memset(dummy_w, 0.0)
            for i in range(warmup):
                wps = psum_pool.tile([P, S2], F32, tag="ps", name=f"warm{i}")
                nc.tensor.matmul(wps[:, 0:NS], lhsT=dummy_x, rhs=dummy_w,
                                 start=True, stop=True)

        # xn loads / bias computations are issued one per group below;
        # r_bias goes ahead of the group's multiply in the DVE FIFO so a
        # multiply blocked on ACT can't delay later bias computations.
        sched = {"xn": xn_early, "rb": xn_early}

        def schedule_rbias():
            if sched["xn"] < NB:
                load_xn(sched["xn"])
                sched["xn"] += 1
            if sched["rb"] < NB:
                r_bias(sched["rb"])
                sched["rb"] += 1

        def drain(ps, nb, h, last):
            # the final psum group drains in NS-wide pieces so the last
            # exp->mul->store chain is short
            npc = S2 // NS if last else 1
            pw = S2 // npc
            for pc in range(npc):
                tmp = tmp_pool.tile([P, pw], BF16, tag="tmp")
                nc.scalar.activation(
                    tmp, ps[:, pc * pw:(pc + 1) * pw],
                    func=mybir.ActivationFunctionType.Exp,
                    bias=bias_tiles[nb],
                    scale=float(1.0 / W_SCALE),
                )
                hsl = slice(h * S2 + pc * pw, h * S2 + (pc + 1) * pw)
                o_sb = out_pool.tile([P, pw], BF16, tag="osb")
                nc.vector.tensor_mul(o_sb, tmp, eb[:, hsl])
                oeng = nc.scalar if last else nc.sync
                oeng.dma_start(out[nb * P:(nb + 1) * P, hsl], o_sb)

        # band 0 (row blocks 0..3) sweeps h-banded: the four h=0 psum
        # groups only touch W half 0, giving the W half-1 DMA headroom.
        BAND = 4
        for h in range(SH):
            for nb in range(BAND):
                if h == 0:
                    schedule_rbias()
                ps = psum_pool.tile([P, S2], F32, tag="ps", name=f"ps{nb}_{h}")
                for kp in range(KP):
                    lhsT = x_sb[:, 2 * kp:2 * kp + 2, nb * P:(nb + 1) * P]
                    for c in range(S2 // NS):
                        col0 = h * S2 + c * NS
                        nc.tensor.matmul(
                            ps[:, c * NS:(c + 1) * NS],
                            lhsT=lhsT,
                            rhs=w_sb[:, 2 * kp:2 * kp + 2, col0:col0 + NS],
                            start=(kp == 0),
                            stop=(kp == KP - 1),
                            perf_mode=DR,
                        )
                drain(ps, nb, h, False)

        # remaining row blocks: both psum groups interleaved in one kp
        # sweep so a stationary x block feeds 2*S2/NS matmuls (W is fully
        # resident by now).  The very last group runs its column banks
        # back-to-back so its first ACT piece overlaps the closing matmuls.
        for nb in range(BAND, NB):
            schedule_rbias()
            last_nb = nb == NB - 1
            pss = [psum_pool.tile([P, S2], F32, tag="ps", name=f"ps{nb}_{h}")
                   for h in range(SH)]
            if not last_nb:
                for kp in range(KP):
                    lhsT = x_sb[:, 2 * kp:2 * kp + 2, nb * P:(nb + 1) * P]
                    for h in range(SH):
                        for c in range(S2 // NS):
                            col0 = h * S2 + c * NS
                            nc.tensor.matmul(
                                pss[h][:, c * NS:(c + 1) * NS],
                                lhsT=lhsT,
                                rhs=w_sb[:, 2 * kp:2 * kp + 2, col0:col0 + NS],
                                start=(kp == 0),
                                stop=(kp == KP - 1),
                                perf_mode=DR,
                            )
                for h in range(SH):
                    drain(pss[h], nb, h, False)
            else:
                for h in range(SH):
                    for c in range(S2 // NS):
                        col0 = h * S2 + c * NS
                        for kp in range(KP):
                            nc.tensor.matmul(
                                pss[h][:, c * NS:(c + 1) * NS],
                                lhsT=x_sb[:, 2 * kp:2 * kp + 2,
                                          nb * P:(nb + 1) * P],
                                rhs=w_sb[:, 2 * kp:2 * kp + 2, col0:col0 + NS],
                                start=(kp == 0),
                                stop=(kp == KP - 1),
                                perf_mode=DR,
                            )
                    drain(pss[h], nb, h, h == SH - 1)

    nc.compile()
    return nc


_NC_CACHE = {}


def _get_nc(**kwargs):
    key = tuple(sorted(kwargs.items()))
    if key not in _NC_CACHE:
        _NC_CACHE[key] = build_nc(**kwargs)
    return _NC_CACHE[key]


def make_in_maps(x, W, b):
    import ml_dtypes
    fp8 = ml_dtypes.float8_e4m3
    bf16 = ml_dtypes.bfloat16
    wT = np.ascontiguousarray((W.T * W_SCALE).astype(fp8))
    b = np.ascontiguousarray(np.exp(b.astype(np.float64)).astype(bf16))
    in_maps = []
    for i in range(N_CORES):
        xs = x[i * NC_FULL:(i + 1) * NC_FULL]
        in_maps.append({
            "xT": np.ascontiguousarray(xs.T.astype(fp8)),
            "xn": np.ascontiguousarray(xs.astype(np.float16)),
            "w": wT,
            "bias": b,
        })
    return in_maps


def run_hw(x, W, b, trace=False, **build_kwargs):
    """Run on 8 NeuronCores; returns (out [N, S] f32, BassKernelResults)."""
    from concourse.bass_utils import run_bass_kernel_spmd
    from concourse.bass_interp import get_hw_module

    nc = _get_nc(**build_kwargs)
    in_maps = make_in_maps(x, W, b)
    old_m = nc.m
    nc.m = get_hw_module(nc.m)
    try:
        res = run_bass_kernel_spmd(
            nc, in_maps, core_ids=list(range(N_CORES)), trace=trace)
    finally:
        nc.m = old_m
    out = np.concatenate(
        [res.results[i]["out"] for i in range(N_CORES)], axis=0)
    return out.astype(np.float32), res


def kernel(x, W, b):
    out, _ = run_hw(x, W, b, trace=False)
    return out
